# revision 18
# baseline (speedup 1.0000x reference)
"""GCN + SortPooling (DGCNN) Trainium2 Bass kernel — v2.

Sharding: 250 graphs split across 8 NeuronCores (2x32 + 6x31 graphs).
Message passing uses Ã h = dis ⊙ (S + h'), h' = dis ⊙ (hW),
S[v] = Σ_{e→v} h'[src_e], dis = rsqrt(deg+1).

v1 gathered h'[src] with one indirect DMA per 128 edges (12k
instructions × ~1µs SWDGE fixed overhead = the whole runtime). v2
instead uses bulk dma_gather (InstDMAGatherAnt): per pass ~60
instructions each gathering up to 8192 edge rows (256B elements from a
64ch-padded table; int16 indices restrict each instruction to one of 4
windows of 25728 rows). Edges are laid out edge-major, sorted by
(window, dst block); the segment-sum is done on the PE: for each
128-edge column a one-hot matrix M (built on DVE via is_equal against
an iota row) maps edges to dst slots, and matmuls accumulate S per
(block, window) in PSUM, then per block in SBUF across windows.
Self-loop h' and dis scaling are applied in the per-block epilogue as
in v1. Sort-pool: per-graph top-100 via repeated DVE
max8/max_index/match_replace; head (conv1d/dense) on PE.
"""
import os
import sys

sys.path.insert(0, "/opt/trn_rl_repo")

_ABL = os.environ.get("KABL", "")

import numpy as np

import concourse.bass as bass
import concourse.bacc as bacc
import concourse.mybir as mybir
import concourse.tile as tile
from concourse.masks import make_identity
from concourse.bass_utils import run_bass_kernel_spmd

F32 = mybir.dt.float32
BF16 = mybir.dt.bfloat16
I16 = mybir.dt.int16
ALU = mybir.AluOpType
ACT = mybir.ActivationFunctionType

NUM_NODES = 100000
N_PER = 400
NUM_GRAPHS = 250
K = 100
N_CORES = 8
GRAPHS_PER_CORE = [32, 32, 31, 31, 31, 31, 31, 31]
G_MAX = 32
N_LOC = G_MAX * N_PER            # 12800
NODE_BASE = np.cumsum([0] + [g * N_PER for g in GRAPHS_PER_CORE])[:-1]
N_REAL = [g * N_PER for g in GRAPHS_PER_CORE]
NBLK = N_LOC // 128              # 100
CH = 32
XC = 112                         # padded concat channels (97 used)
NWIN = 4                         # gather windows (2 cores' rows each)
WIN_REAL = 2 * N_LOC             # 25600 real rows per window
WIN_PAD = WIN_REAL + 128         # + zero rows
E_PAD = 64                       # table row padded to 64 f32 = 256B
CHUNK_COLS = 8                   # gather chunk = 8 cols = 1024 idxs
                                 # (SWDGE ring: one dma_gather must stay
                                 # <= ~64 ring entries = 1024 idxs)
MB = 4                           # M-matrix build batch (columns)


def host_prep(edge_index):
    """Edge-major gather layout. Returns per-core tensors + the
    SPMD-uniform compile-time structure."""
    src = edge_index[0].astype(np.int64)
    dst = edge_index[1].astype(np.int64)
    deg = np.bincount(dst, minlength=NUM_NODES).astype(np.float32) + 1.0

    # global node -> (table row, window, in-window idx); natural order
    core_of = np.searchsorted(NODE_BASE, np.arange(NUM_NODES),
                              side="right") - 1
    table_row = core_of * N_LOC + (np.arange(NUM_NODES) - NODE_BASE[core_of])
    win = table_row // WIN_REAL
    inwin = table_row - win * WIN_REAL

    # per-core edge groups by (window, dst block)
    per_core = []            # c -> dict[(q, b)] = (inwin_src, dstmod)
    counts = np.zeros((N_CORES, NWIN, NBLK), np.int64)
    for c in range(N_CORES):
        lo, hi = NODE_BASE[c], NODE_BASE[c] + N_REAL[c]
        m = (dst >= lo) & (dst < hi)
        s_c, d_c = src[m], dst[m]
        local_d = d_c - lo
        q_c = win[s_c]
        b_c = local_d // 128
        order = np.lexsort((local_d, b_c, q_c))
        s_c, local_d, q_c, b_c = (s_c[order], local_d[order],
                                  q_c[order], b_c[order])
        iw = inwin[s_c]
        dm = local_d % 128
        # group boundaries
        key = q_c * NBLK + b_c
        groups = {}
        bounds = np.searchsorted(key, np.arange(NWIN * NBLK + 1))
        for g in range(NWIN * NBLK):
            a, b_ = bounds[g], bounds[g + 1]
            if b_ > a:
                groups[(g // NBLK, g % NBLK)] = (iw[a:b_], dm[a:b_])
        per_core.append(groups)
        np.add.at(counts, (c, q_c, b_c), 1)

    # SPMD-uniform columns per (q, b)
    ncol = np.ceil(counts / 128).max(axis=0).astype(np.int64)  # [NWIN, NBLK]

    # chunk structure per window
    chunks = []   # (q, n_cols) per gather instruction, in stream order
    colmeta = []  # per column: (b, start, stop)
    for q in range(NWIN):
        cq = 0
        for b in range(NBLK):
            n = int(ncol[q, b])
            for j in range(n):
                colmeta.append((q, b, j == 0, j == n - 1))
            cq += n
        while cq > 0:
            take = min(CHUNK_COLS, cq)
            chunks.append((q, take))
            cq -= take
    total_cols = len(colmeta)

    # per-core idx + dstcol streams
    idx_cores, dc_cores = [], []
    for c in range(N_CORES):
        groups = per_core[c]
        idx_flat = np.empty(total_cols * 128, np.int16)
        dc_flat = np.full((128, total_cols), -1.0, np.float32)
        pos = 0
        for q in range(NWIN):
            for b in range(NBLK):
                n = int(ncol[q, b])
                if n == 0:
                    continue
                cap = n * 128
                iw, dm = groups.get((q, b), (np.empty(0, np.int64),
                                             np.empty(0, np.int64)))
                e = len(iw)
                seg = np.full(cap, 0, np.int64)
                seg[:e] = iw
                pad_pos = np.arange(e, cap)
                seg[e:] = WIN_REAL + (pad_pos % 128)
                idx_flat[pos * 128:(pos + n) * 128] = seg.astype(np.int16)
                dseg = np.full(cap, -1.0, np.float32)
                dseg[:e] = dm.astype(np.float32)
                dc_flat[:, pos:pos + n] = dseg.reshape(n, 128).T
                pos += n
        assert pos == total_cols
        # wrap idx per chunk: [16, cols*8] tiled to [128, cols*8]
        blocks = []
        cpos = 0
        for (q, ncols) in chunks:
            seg = idx_flat[cpos * 128:(cpos + ncols) * 128]
            w = seg.reshape(ncols * 8, 16).T        # [16, ncols*8]
            blocks.append(np.tile(w, (8, 1)))
            cpos += ncols
        idx_cores.append(np.ascontiguousarray(np.concatenate(blocks, axis=1)))
        dc_cores.append(np.ascontiguousarray(dc_flat))

    deg_cores = []
    for c in range(N_CORES):
        d = np.ones(N_LOC, np.float32)
        d[: N_REAL[c]] = deg[NODE_BASE[c]:NODE_BASE[c] + N_REAL[c]]
        deg_cores.append(d)
    struct = (tuple(int(x) for x in ncol.ravel()),
              tuple(chunks))
    return idx_cores, dc_cores, deg_cores, ncol, chunks, colmeta, struct


def dma_gather_raw(gp, out_ap, in_ap, idxs_ap, num_idxs, elem_size,
                   elem_step, queue_num=0):
    """dma_gather with elem_size*dtype < 256B (measured ~2.5x faster
    than 256B elements; bass's %256 assert is a transpose-mode
    restriction that does not apply to non-transpose HW behavior)."""
    stride_bytes_256 = (elem_step * 4) // 256
    _in_ap = gp.lower_ap_dma(in_ap, for_custom_bir_dma=True)
    _idxs_ap = gp.lower_ap(idxs_ap)
    _out_ap = gp.lower_ap(out_ap)
    return gp.add_instruction(mybir.InstDMAGatherAnt(
        name=gp.bass.get_next_instruction_name(),
        ins=[*_in_ap, _idxs_ap,
             gp.lower_val_access(gp.to_reg(num_idxs))],
        outs=[_out_ap], transpose=False, num_idxs=num_idxs,
        elem_size=elem_size, stride_bytes_256=stride_bytes_256,
        gen_mode=0, single_packet=True, queue_num=queue_num,
        sbuf_tokens_per_rank=0, sbuf_free_dim_per_rank=0,
        sbuf_free_dim_pad_per_rank=0, sbuf_byte_offset=0))


def build_kernel(ncol, chunks, colmeta):
    total_cols = len(colmeta)
    nc = bacc.Bacc("TRN2", target_bir_lowering=False, debug=False,
                   num_devices=N_CORES, num_swdge_queues=4)

    xt_d = nc.dram_tensor("xt", [128, N_LOC], F32, kind="ExternalInput")
    idx_d = nc.dram_tensor("gidx", [128, total_cols * 8], I16,
                           kind="ExternalInput")
    dc_d = nc.dram_tensor("dcol", [128, total_cols], F32,
                          kind="ExternalInput")
    degp_d = nc.dram_tensor("degp", [128, NBLK], F32, kind="ExternalInput")
    iota_d = nc.dram_tensor("iota", [128, 128], F32, kind="ExternalInput")
    w1_d = nc.dram_tensor("w1", [128, CH], F32, kind="ExternalInput")
    w2_d = nc.dram_tensor("w2", [CH, CH], F32, kind="ExternalInput")
    w3_d = nc.dram_tensor("w3", [CH, CH], F32, kind="ExternalInput")
    w4_d = nc.dram_tensor("w4", [CH, 1], F32, kind="ExternalInput")
    bst_d = nc.dram_tensor("bst", [CH, 4], F32, kind="ExternalInput")
    gb_d = nc.dram_tensor("gbase", [G_MAX, 1], F32, kind="ExternalInput")
    cw1_d = nc.dram_tensor("cw1t", [97, 128], F32, kind="ExternalInput")
    cb1_d = nc.dram_tensor("cb1", [128, 1], F32, kind="ExternalInput")
    cw2_d = nc.dram_tensor("cw2t", [128, 5 * 64], F32, kind="ExternalInput")
    cb2_d = nc.dram_tensor("cb2", [64, 1], F32, kind="ExternalInput")
    fw1_d = nc.dram_tensor("fw1p", [64, 46 * 128], F32, kind="ExternalInput")
    fb1_d = nc.dram_tensor("fb1", [128, 1], F32, kind="ExternalInput")
    fw3_d = nc.dram_tensor("fw3t", [128, 2], F32, kind="ExternalInput")
    fb3_d = nc.dram_tensor("fb3", [2, 1], F32, kind="ExternalInput")
    out_d = nc.dram_tensor("out", [1, 2 * G_MAX], F32, kind="ExternalOutput")

    U32 = mybir.dt.uint32

    with tile.TileContext(nc) as tc:
        with tc.tile_pool(name="dram", bufs=1, space="DRAM") as dpool, \
             tc.tile_pool(name="const", bufs=1) as cpool, \
             tc.tile_pool(name="big", bufs=1) as bigpool, \
             tc.tile_pool(name="msg", bufs=3) as msgpool, \
             tc.tile_pool(name="mmat", bufs=3) as mpool, \
             tc.tile_pool(name="io", bufs=2) as iopool, \
             tc.tile_pool(name="work", bufs=2) as wpool, \
             tc.tile_pool(name="psA", bufs=2, space="PSUM") as psA, \
             tc.tile_pool(name="psB", bufs=2, space="PSUM") as psB, \
             tc.tile_pool(name="psS", bufs=2, space="PSUM") as psS:

            bounce = dpool.tile([N_LOC, CH], F32)
            tabC = dpool.tile([N_CORES * N_LOC, CH], F32)
            tabD = dpool.tile([NWIN * WIN_PAD, E_PAD], F32)
            xcd = dpool.tile([N_LOC, XC], F32)

            def load_const(dram, shape, dtype=F32):
                t = cpool.tile(shape, dtype, tag=f"c_{dram.name}")
                nc.sync.dma_start(out=t[:], in_=dram[:])
                return t

            w1 = load_const(w1_d, [128, CH])
            w2 = load_const(w2_d, [CH, CH])
            w3 = load_const(w3_d, [CH, CH])
            w4 = load_const(w4_d, [CH, 1])
            bst = load_const(bst_d, [CH, 4])
            gbase = load_const(gb_d, [G_MAX, 1])
            iota = load_const(iota_d, [128, 128])
            cw1t = load_const(cw1_d, [97, 128])
            cb1 = load_const(cb1_d, [128, 1])
            cw2t = load_const(cw2_d, [128, 5 * 64])
            cb2 = load_const(cb2_d, [64, 1])
            fw1p = load_const(fw1_d, [64, 46 * 128])
            fb1 = load_const(fb1_d, [128, 1])
            fw3t = load_const(fw3_d, [128, 2])
            fb3 = load_const(fb3_d, [2, 1])

            ident = cpool.tile([128, 128], F32)
            make_identity(nc, ident[:])

            # zero rows of each gather window (cols 0:CH suffice, but
            # clear the full padded row once)
            zrow = cpool.tile([128, E_PAD], F32)
            nc.vector.memset(zrow[:], 0.0)
            for q in range(NWIN):
                nc.sync.dma_start(
                    out=tabD[q * WIN_PAD + WIN_REAL:(q + 1) * WIN_PAD, :],
                    in_=zrow[:])

            disp = load_const(degp_d, [128, NBLK])
            nc.vector.reciprocal(out=disp[:], in_=disp[:])
            nc.scalar.activation(out=disp[:], in_=disp[:], func=ACT.Sqrt)

            hrows = bigpool.tile([128, NBLK, CH], F32)   # h' rows (scaled)
            xct = bigpool.tile([XC, N_LOC], F32)
            accR = bigpool.tile([128, NBLK, CH], F32)    # S accumulator
            gixA = bigpool.tile([128, total_cols * 8], I16)
            nc.sync.dma_start(out=gixA[:], in_=idx_d[:])
            dcolA = bigpool.tile([128, total_cols], F32)
            nc.sync.dma_start(out=dcolA[:], in_=dc_d[:])

            # ---------------- pass prologue ----------------
            def prologue(pass_i):
                for ci in range(50):
                    sl = slice(ci * 256, (ci + 1) * 256)
                    tt = wpool.tile([CH, 256], F32, tag="htc")
                    if pass_i == 0:
                        xt_sb = wpool.tile([128, 256], F32, tag="xtc")
                        nc.sync.dma_start(out=xt_sb[:], in_=xt_d[:, sl])
                        ps = psA.tile([CH, 256], F32, space="PSUM", tag="ps")
                        nc.tensor.matmul(out=ps[:], lhsT=w1[:], rhs=xt_sb[:],
                                         start=True, stop=True)
                        nc.vector.tensor_copy(out=tt[:], in_=ps[:])
                    elif pass_i == 1:
                        ps = psA.tile([CH, 256], F32, space="PSUM", tag="ps")
                        nc.tensor.matmul(out=ps[:], lhsT=w2[:],
                                         rhs=xct[0:CH, sl],
                                         start=True, stop=True)
                        nc.vector.tensor_copy(out=tt[:], in_=ps[:])
                    else:
                        nc.vector.tensor_copy(out=tt[:], in_=xct[CH:2 * CH, sl])
                    for j in range(2):
                        c = 2 * ci + j
                        pst = psA.tile([128, CH], F32, space="PSUM", tag="ps")
                        nc.tensor.transpose(out=pst[:],
                                            in_=tt[:, 128 * j:128 * (j + 1)],
                                            identity=ident[0:CH, 0:CH])
                        nc.vector.tensor_scalar_mul(out=hrows[:, c, :],
                                                    in0=pst[:],
                                                    scalar1=disp[:, c:c + 1])
                nc.sync.dma_start(
                    out=bounce[:].rearrange("(b p) c -> p b c", p=128),
                    in_=hrows[:])

            # ---------------- epilogue for one block ----------------
            def block_epilogue(pass_i, b):
                sl = slice(b * 128, (b + 1) * 128)
                t1 = wpool.tile([128, CH], F32, tag="fl")
                nc.vector.tensor_tensor(out=t1[:], in0=accR[:, b, :],
                                        in1=hrows[:, b, :], op=ALU.add)
                nc.vector.tensor_scalar_mul(out=t1[:], in0=t1[:],
                                            scalar1=disp[:, b:b + 1])
                psu = psA.tile([CH, 128], F32, space="PSUM", tag="ps")
                nc.tensor.transpose(out=psu[:], in_=t1[:], identity=ident[:])
                if pass_i == 0:
                    nc.scalar.activation(out=xct[0:CH, sl], in_=psu[:],
                                         func=ACT.Tanh, bias=bst[:, 0:1])
                elif pass_i == 1:
                    nc.scalar.activation(out=xct[CH:2 * CH, sl], in_=psu[:],
                                         func=ACT.Tanh, bias=bst[:, 1:2])
                else:
                    ut = wpool.tile([CH, 128], F32, tag="ut")
                    nc.vector.tensor_copy(out=ut[:], in_=psu[:])
                    ps3 = psA.tile([CH, 128], F32, space="PSUM", tag="ps")
                    nc.tensor.matmul(out=ps3[:], lhsT=w3[:], rhs=ut[:],
                                     start=True, stop=True)
                    nc.scalar.activation(out=xct[2 * CH:3 * CH, sl],
                                         in_=ps3[:], func=ACT.Tanh,
                                         bias=bst[:, 2:3])
                    ps4 = psA.tile([1, 128], F32, space="PSUM", tag="ps")
                    nc.tensor.matmul(out=ps4[:], lhsT=w4[:], rhs=ut[:],
                                     start=True, stop=True)
                    nc.scalar.activation(out=xct[96:97, sl], in_=ps4[:],
                                         func=ACT.Tanh, bias=bst[0:1, 3:4])

            # ---------------- edge phase ----------------
            def edge_phase(pass_i):
                nc.gpsimd.collective_compute(
                    "AllGather", ALU.bypass,
                    replica_groups=[list(range(N_CORES))],
                    ins=[bounce[:]], outs=[tabC[:]])
                # restride 32ch rows -> 64ch padded windows
                for q in range(NWIN):
                    nc.sync.dma_start(
                        out=tabD[q * WIN_PAD:q * WIN_PAD + WIN_REAL, 0:CH],
                        in_=tabC[q * WIN_REAL:(q + 1) * WIN_REAL, :])

                nc.vector.memset(accR[:].rearrange("p b c -> p (b c)"), 0.0)

                seen = set()
                col = 0          # global column index
                mcur = None
                for ic, (q, ncols) in enumerate(chunks):
                    n = ncols * 128
                    mblk = msgpool.tile([128, CHUNK_COLS, CH], F32,
                                        tag="m")
                    if _ABL not in ("nogather", "skel"):
                        dma_gather_raw(
                            nc.gpsimd, mblk[:, 0:ncols, :],
                            tabD[q * WIN_PAD:(q + 1) * WIN_PAD, :],
                            gixA[:, col * 8:(col + ncols) * 8], n, CH, E_PAD,
                            queue_num=ic % 4)
                    if _ABL not in ("nomm", "skel"):
                        hl = msgpool.tile([128, CHUNK_COLS, 2, CH], BF16,
                                          tag="hl")
                        nc.vector.tensor_copy(out=hl[:, 0:ncols, 0, :],
                                              in_=mblk[:, 0:ncols, :])
                        nc.vector.tensor_tensor(out=mblk[:, 0:ncols, :],
                                                in0=mblk[:, 0:ncols, :],
                                                in1=hl[:, 0:ncols, 0, :],
                                                op=ALU.subtract)
                        nc.vector.tensor_copy(out=hl[:, 0:ncols, 1, :],
                                              in_=mblk[:, 0:ncols, :])
                    for j in range(ncols if _ABL not in ("nomm", "skel") else 0):
                        (qq, b, st, sp) = colmeta[col + j]
                        if j % MB == 0:
                            nb = min(MB, ncols - j)
                            mcur = mpool.tile([128, MB, 128], BF16, tag="mm")
                            nc.vector.tensor_tensor(
                                out=mcur[:, 0:nb, :],
                                in0=dcolA[:, col + j:col + j + nb].rearrange(
                                    "p (k o) -> p k o", o=1).to_broadcast(
                                    [128, nb, 128]),
                                in1=iota[:].rearrange(
                                    "p (o v) -> p o v", o=1).to_broadcast(
                                    [128, nb, 128]),
                                op=ALU.is_equal)
                        if st:
                            ps = psS.tile([128, 2 * CH], F32, space="PSUM",
                                          tag="s")
                            cur_ps = ps
                        nc.tensor.matmul(out=cur_ps[:],
                                         lhsT=mcur[:, j % MB, :],
                                         rhs=hl[:, j, :, :].rearrange(
                                             "p h c -> p (h c)"),
                                         start=st, stop=sp)
                        if sp:
                            if b in seen:
                                nc.vector.tensor_tensor(out=accR[:, b, :],
                                                        in0=accR[:, b, :],
                                                        in1=cur_ps[:, 0:CH],
                                                        op=ALU.add)
                            else:
                                nc.vector.tensor_copy(out=accR[:, b, :],
                                                      in_=cur_ps[:, 0:CH])
                                seen.add(b)
                            nc.vector.tensor_tensor(out=accR[:, b, :],
                                                    in0=accR[:, b, :],
                                                    in1=cur_ps[:, CH:2 * CH],
                                                    op=ALU.add)
                    col += ncols
                for b in range(NBLK):
                    block_epilogue(pass_i, b)

            for pass_i in range(3):
                prologue(pass_i)
                edge_phase(pass_i)

            # ---------------- xct -> xcd (rows in DRAM) ----------------
            for c in range(NBLK):
                pst = psA.tile([128, XC], F32, space="PSUM", tag="ps")
                nc.tensor.transpose(out=pst[:],
                                    in_=xct[:, c * 128:(c + 1) * 128],
                                    identity=ident[0:XC, 0:XC])
                xr = wpool.tile([128, XC], F32, tag="xr")
                nc.vector.tensor_copy(out=xr[:], in_=pst[:])
                nc.sync.dma_start(out=xcd[c * 128:(c + 1) * 128, :], in_=xr[:])

            # ---------------- sort pooling ----------------
            keys = cpool.tile([G_MAX, N_PER], F32)
            nc.sync.dma_start(
                out=keys[:],
                in_=xcd[:, 96:97].rearrange("(g k) c -> g k c", k=N_PER))
            vals = cpool.tile([G_MAX, N_PER], F32)
            nc.vector.tensor_copy(out=vals[:], in_=keys[:])
            idxs = cpool.tile([G_MAX, 104], U32)
            for r in range(13):
                m8 = wpool.tile([G_MAX, 8], F32, tag="m8")
                nc.vector.max(out=m8[:], in_=vals[:])
                nc.vector.max_index(out=idxs[:, 8 * r:8 * r + 8],
                                    in_max=m8[:], in_values=keys[:])
                nc.vector.match_replace(out=vals[:], in_to_replace=m8[:],
                                        in_values=vals[:], imm_value=-1e30)
            idxf = cpool.tile([G_MAX, 104], F32)
            nc.vector.tensor_copy(out=idxf[:], in_=idxs[:])
            nc.vector.tensor_scalar(out=idxf[:], in0=idxf[:],
                                    scalar1=gbase[:], scalar2=None,
                                    op0=ALU.add)
            psi = psA.tile([104, G_MAX], F32, space="PSUM", tag="ps")
            nc.tensor.transpose(out=psi[:], in_=idxf[:],
                                identity=ident[0:G_MAX, 0:G_MAX])
            idxT = cpool.tile([128, G_MAX], mybir.dt.int32)
            nc.vector.memset(idxT[:], 0)
            nc.vector.tensor_copy(out=idxT[0:104, :], in_=psi[:])

            # ---------------- head ----------------
            p1all = bigpool.tile([128, 50 * G_MAX], F32)
            for g in range(G_MAX):
                xsg = wpool.tile([128, XC], F32, tag="xsg")
                nc.gpsimd.indirect_dma_start(
                    out=xsg[:], out_offset=None, in_=xcd[:],
                    in_offset=bass.IndirectOffsetOnAxis(
                        ap=idxT[:, g:g + 1], axis=0))
                pst = psA.tile([XC, 104], F32, space="PSUM", tag="ps")
                nc.tensor.transpose(out=pst[:], in_=xsg[0:104, :],
                                    identity=ident[0:104, 0:104])
                xsgT = wpool.tile([XC, 104], F32, tag="xsgT")
                nc.vector.tensor_copy(out=xsgT[:], in_=pst[:])
                c1 = psB.tile([128, K], F32, space="PSUM", tag="acc")
                nc.tensor.matmul(out=c1[:], lhsT=cw1t[:],
                                 rhs=xsgT[0:97, 0:K], start=True, stop=True)
                mp = wpool.tile([128, 50], F32, tag="mp")
                nc.vector.tensor_copy(out=mp[:], in_=c1[:, 0:K:2])
                nc.vector.tensor_tensor(out=mp[:], in0=mp[:],
                                        in1=c1[:, 1:K:2], op=ALU.max)
                nc.scalar.activation(out=p1all[:, 50 * g:50 * g + 50],
                                     in_=mp[:], func=ACT.Relu, bias=cb1[:])

            c2all = bigpool.tile([64, 46 * G_MAX], F32)
            for g0 in range(0, G_MAX, 8):
                gn = min(8, G_MAX - g0)
                ps = psA.tile([64, 46 * gn], F32, space="PSUM", tag="ps")
                for t in range(5):
                    rhs = p1all[:].rearrange("p (g x) -> p g x", x=50)[
                        :, g0:g0 + gn, t:t + 46]
                    nc.tensor.matmul(out=ps[:],
                                     lhsT=cw2t[:, 64 * t:64 * (t + 1)],
                                     rhs=rhs, start=(t == 0), stop=(t == 4))
                nc.scalar.activation(out=c2all[:, 46 * g0:46 * (g0 + gn)],
                                     in_=ps[:], func=ACT.Relu, bias=cb2[:])

            hps = psA.tile([128, G_MAX], F32, space="PSUM", tag="ps")
            for p in range(46):
                rhs = c2all[:].rearrange("p (g x) -> p g x", x=46)[:, :, p:p + 1]
                nc.tensor.matmul(out=hps[:],
                                 lhsT=fw1p[:, 128 * p:128 * (p + 1)],
                                 rhs=rhs, start=(p == 0), stop=(p == 45))
            hsb = cpool.tile([128, G_MAX], F32)
            nc.scalar.activation(out=hsb[:], in_=hps[:], func=ACT.Relu,
                                 bias=fb1[:])

            lps = psA.tile([2, G_MAX], F32, space="PSUM", tag="ps")
            nc.tensor.matmul(out=lps[:], lhsT=fw3t[:], rhs=hsb[:],
                             start=True, stop=True)
            lg2 = cpool.tile([2, G_MAX], F32)
            nc.vector.tensor_copy(out=lg2[:], in_=lps[:])
            lgf = cpool.tile([1, 2 * G_MAX], F32)
            nc.sync.dma_start(
                out=lgf[:].rearrange("p (c g) -> p c g", c=2), in_=lg2[:])
            fb3f = cpool.tile([1, 2], F32)
            nc.sync.dma_start(out=fb3f[:], in_=fb3_d[:].rearrange("c x -> x c"))
            G = G_MAX
            for c in range(2):
                nc.vector.tensor_tensor(
                    out=lgf[:, c * G:(c + 1) * G],
                    in0=lgf[:, c * G:(c + 1) * G],
                    in1=fb3f[:, c:c + 1].to_broadcast([1, G]), op=ALU.add)
            mx = cpool.tile([1, G], F32)
            nc.vector.tensor_tensor(out=mx[:], in0=lgf[:, 0:G],
                                    in1=lgf[:, G:2 * G], op=ALU.max)
            dd = cpool.tile([1, 2 * G], F32)
            for c in range(2):
                nc.vector.tensor_tensor(out=dd[:, c * G:(c + 1) * G],
                                        in0=lgf[:, c * G:(c + 1) * G],
                                        in1=mx[:], op=ALU.subtract)
            ee = cpool.tile([1, 2 * G], F32)
            nc.scalar.activation(out=ee[:], in_=dd[:], func=ACT.Exp)
            ss = cpool.tile([1, G], F32)
            nc.vector.tensor_tensor(out=ss[:], in0=ee[:, 0:G],
                                    in1=ee[:, G:2 * G], op=ALU.add)
            nc.scalar.activation(out=ss[:], in_=ss[:], func=ACT.Ln)
            res = cpool.tile([1, 2 * G], F32)
            for c in range(2):
                nc.vector.tensor_tensor(out=res[:, c * G:(c + 1) * G],
                                        in0=dd[:, c * G:(c + 1) * G],
                                        in1=ss[:], op=ALU.subtract)
            nc.sync.dma_start(out=out_d[:], in_=res[:])

    nc.compile()
    return nc


_CACHED = {}


def kernel(**inputs):
    x = np.asarray(inputs["x"], np.float32)
    edge_index = np.asarray(inputs["edge_index"])
    (idx_cores, dc_cores, deg_cores, ncol, chunks, colmeta,
     struct) = host_prep(edge_index)

    W = {k: np.asarray(inputs[k], np.float32) for k in
         ("W1", "b1", "W2", "b2", "W3", "b3", "W4", "b4",
          "cw1", "cb1", "cw2", "cb2", "fw1", "fb1", "fw3", "fb3")}

    if _CACHED.get("key") != struct:
        _CACHED["nc"] = build_kernel(ncol, chunks, colmeta)
        _CACHED["key"] = struct
    nc = _CACHED["nc"]

    bs = np.zeros((4, CH), np.float32)
    bs[0], bs[1], bs[2] = W["b1"], W["b2"], W["b3"]
    bs[3, 0] = W["b4"][0]
    bst = np.ascontiguousarray(bs.T)  # [CH, 4]
    cw2t = W["cw2"].transpose(1, 2, 0)            # [ic, t, oc]
    cw2t = np.ascontiguousarray(cw2t.reshape(128, 5 * 64))
    fw1p = W["fw1"].reshape(128, 64, 46).transpose(1, 2, 0)  # [ch,pos,hid]
    fw1p = np.ascontiguousarray(fw1p.reshape(64, 46 * 128))
    shared = {
        "w1": W["W1"], "w2": W["W2"], "w3": W["W3"], "w4": W["W4"],
        "bst": bst,
        "gbase": (np.arange(G_MAX, dtype=np.float32) * N_PER)[:, None],
        "iota": np.tile(np.arange(128, dtype=np.float32)[None, :], (128, 1)),
        "cw1t": np.ascontiguousarray(W["cw1"][:, 0, :].T),
        "cb1": W["cb1"][:, None],
        "cw2t": cw2t, "cb2": W["cb2"][:, None],
        "fw1p": fw1p, "fb1": W["fb1"][:, None],
        "fw3t": np.ascontiguousarray(W["fw3"].T), "fb3": W["fb3"][:, None],
    }

    in_maps = []
    for c in range(N_CORES):
        lo = NODE_BASE[c]
        xc_ = np.zeros((N_LOC, 128), np.float32)
        xc_[: N_REAL[c]] = x[lo: lo + N_REAL[c]]
        m = {
            "xt": np.ascontiguousarray(xc_.T),
            "gidx": idx_cores[c],
            "dcol": dc_cores[c],
            "degp": np.ascontiguousarray(deg_cores[c].reshape(NBLK, 128).T),
        }
        m.update(shared)
        in_maps.append({k: np.ascontiguousarray(
            v, dtype=np.int16 if k == "gidx" else np.float32)
            for k, v in m.items()})

    _CACHED["in_maps"] = in_maps
    res = run_bass_kernel_spmd(nc, in_maps, core_ids=list(range(N_CORES)))

    out = np.zeros((NUM_GRAPHS, 2), np.float32)
    gb = 0
    for c in range(N_CORES):
        g = GRAPHS_PER_CORE[c]
        out[gb:gb + g] = res.results[c]["out"].reshape(2, G_MAX)[:, :g].T
        gb += g
    return out


# revision 20
# speedup vs baseline: 1.2877x; 1.2877x over previous
"""GCN + SortPooling (DGCNN) Trainium2 Bass kernel — v2.

Sharding: 250 graphs split across 8 NeuronCores (2x32 + 6x31 graphs).
Message passing uses Ã h = dis ⊙ (S + h'), h' = dis ⊙ (hW),
S[v] = Σ_{e→v} h'[src_e], dis = rsqrt(deg+1).

v1 gathered h'[src] with one indirect DMA per 128 edges (12k
instructions × ~1µs SWDGE fixed overhead = the whole runtime). v2
instead uses bulk dma_gather (InstDMAGatherAnt): per pass ~60
instructions each gathering up to 8192 edge rows (256B elements from a
64ch-padded table; int16 indices restrict each instruction to one of 4
windows of 25728 rows). Edges are laid out edge-major, sorted by
(window, dst block); the segment-sum is done on the PE: for each
128-edge column a one-hot matrix M (built on DVE via is_equal against
an iota row) maps edges to dst slots, and matmuls accumulate S per
(block, window) in PSUM, then per block in SBUF across windows.
Self-loop h' and dis scaling are applied in the per-block epilogue as
in v1. Sort-pool: per-graph top-100 via repeated DVE
max8/max_index/match_replace; head (conv1d/dense) on PE.
"""
import os
import sys

sys.path.insert(0, "/opt/trn_rl_repo")

_ABL = os.environ.get("KABL", "")

import numpy as np

import concourse.bass as bass
import concourse.bacc as bacc
import concourse.mybir as mybir
import concourse.tile as tile
from concourse.masks import make_identity
from concourse.bass_utils import run_bass_kernel_spmd

F32 = mybir.dt.float32
BF16 = mybir.dt.bfloat16
I16 = mybir.dt.int16
ALU = mybir.AluOpType
ACT = mybir.ActivationFunctionType

NUM_NODES = 100000
N_PER = 400
NUM_GRAPHS = 250
K = 100
N_CORES = 8
GRAPHS_PER_CORE = [32, 32, 31, 31, 31, 31, 31, 31]
G_MAX = 32
N_LOC = G_MAX * N_PER            # 12800
NODE_BASE = np.cumsum([0] + [g * N_PER for g in GRAPHS_PER_CORE])[:-1]
N_REAL = [g * N_PER for g in GRAPHS_PER_CORE]
NBLK = N_LOC // 128              # 100
CH = 32
XC = 112                         # padded concat channels (97 used)
NWIN = 4                         # gather windows (2 cores' rows each)
WIN_REAL = 2 * N_LOC             # 25600 real rows per window
WIN_PAD = WIN_REAL + 128         # + zero rows
E_PAD = 64                       # table row padded to 64 f32 = 256B
CHUNK_COLS = 8                   # gather chunk = 8 cols = 1024 idxs
                                 # (SWDGE ring: one dma_gather must stay
                                 # <= ~64 ring entries = 1024 idxs)
MB = 8                           # M-matrix build batch (columns)


def host_prep(edge_index):
    """Edge-major gather layout. Returns per-core tensors + the
    SPMD-uniform compile-time structure."""
    src = edge_index[0].astype(np.int64)
    dst = edge_index[1].astype(np.int64)
    deg = np.bincount(dst, minlength=NUM_NODES).astype(np.float32) + 1.0

    # global node -> (table row, window, in-window idx); natural order
    core_of = np.searchsorted(NODE_BASE, np.arange(NUM_NODES),
                              side="right") - 1
    table_row = core_of * N_LOC + (np.arange(NUM_NODES) - NODE_BASE[core_of])
    win = table_row // WIN_REAL
    inwin = table_row - win * WIN_REAL

    # per-core edge groups by (window, dst block)
    per_core = []            # c -> dict[(q, b)] = (inwin_src, dstmod)
    counts = np.zeros((N_CORES, NWIN, NBLK), np.int64)
    for c in range(N_CORES):
        lo, hi = NODE_BASE[c], NODE_BASE[c] + N_REAL[c]
        m = (dst >= lo) & (dst < hi)
        s_c, d_c = src[m], dst[m]
        local_d = d_c - lo
        q_c = win[s_c]
        b_c = local_d // 128
        order = np.lexsort((local_d, b_c, q_c))
        s_c, local_d, q_c, b_c = (s_c[order], local_d[order],
                                  q_c[order], b_c[order])
        iw = inwin[s_c]
        dm = local_d % 128
        # group boundaries
        key = q_c * NBLK + b_c
        groups = {}
        bounds = np.searchsorted(key, np.arange(NWIN * NBLK + 1))
        for g in range(NWIN * NBLK):
            a, b_ = bounds[g], bounds[g + 1]
            if b_ > a:
                groups[(g // NBLK, g % NBLK)] = (iw[a:b_], dm[a:b_])
        per_core.append(groups)
        np.add.at(counts, (c, q_c, b_c), 1)

    # SPMD-uniform columns per (q, b)
    ncol = np.ceil(counts / 128).max(axis=0).astype(np.int64)  # [NWIN, NBLK]

    # chunk structure per window
    chunks = []   # (q, n_cols) per gather instruction, in stream order
    colmeta = []  # per column: (b, start, stop)
    for q in range(NWIN):
        cq = 0
        for b in range(NBLK):
            n = int(ncol[q, b])
            for j in range(n):
                colmeta.append((q, b, j == 0, j == n - 1))
            cq += n
        while cq > 0:
            take = min(CHUNK_COLS, cq)
            chunks.append((q, take))
            cq -= take
    total_cols = len(colmeta)

    # per-core idx + dstcol streams
    idx_cores, dc_cores = [], []
    for c in range(N_CORES):
        groups = per_core[c]
        idx_flat = np.empty(total_cols * 128, np.int16)
        dc_flat = np.full((128, total_cols), -1.0, np.float32)
        pos = 0
        for q in range(NWIN):
            for b in range(NBLK):
                n = int(ncol[q, b])
                if n == 0:
                    continue
                cap = n * 128
                iw, dm = groups.get((q, b), (np.empty(0, np.int64),
                                             np.empty(0, np.int64)))
                e = len(iw)
                seg = np.full(cap, 0, np.int64)
                seg[:e] = iw
                pad_pos = np.arange(e, cap)
                seg[e:] = WIN_REAL + (pad_pos % 128)
                idx_flat[pos * 128:(pos + n) * 128] = seg.astype(np.int16)
                dseg = np.full(cap, -1.0, np.float32)
                dseg[:e] = dm.astype(np.float32)
                dc_flat[:, pos:pos + n] = dseg.reshape(n, 128).T
                pos += n
        assert pos == total_cols
        # wrap idx per chunk: [16, cols*8] tiled to [128, cols*8]
        blocks = []
        cpos = 0
        for (q, ncols) in chunks:
            seg = idx_flat[cpos * 128:(cpos + ncols) * 128]
            w = seg.reshape(ncols * 8, 16).T        # [16, ncols*8]
            blocks.append(np.tile(w, (8, 1)))
            cpos += ncols
        idx_cores.append(np.ascontiguousarray(np.concatenate(blocks, axis=1)))
        dc_cores.append(np.ascontiguousarray(dc_flat))

    deg_cores = []
    for c in range(N_CORES):
        d = np.ones(N_LOC, np.float32)
        d[: N_REAL[c]] = deg[NODE_BASE[c]:NODE_BASE[c] + N_REAL[c]]
        deg_cores.append(d)
    struct = (tuple(int(x) for x in ncol.ravel()),
              tuple(chunks))
    return idx_cores, dc_cores, deg_cores, ncol, chunks, colmeta, struct


def dma_gather_raw(gp, out_ap, in_ap, idxs_ap, num_idxs, elem_size,
                   elem_step, queue_num=0):
    """dma_gather with elem_size*dtype < 256B (measured ~2.5x faster
    than 256B elements; bass's %256 assert is a transpose-mode
    restriction that does not apply to non-transpose HW behavior)."""
    stride_bytes_256 = (elem_step * 4) // 256
    _in_ap = gp.lower_ap_dma(in_ap, for_custom_bir_dma=True)
    _idxs_ap = gp.lower_ap(idxs_ap)
    _out_ap = gp.lower_ap(out_ap)
    return gp.add_instruction(mybir.InstDMAGatherAnt(
        name=gp.bass.get_next_instruction_name(),
        ins=[*_in_ap, _idxs_ap,
             gp.lower_val_access(gp.to_reg(num_idxs))],
        outs=[_out_ap], transpose=False, num_idxs=num_idxs,
        elem_size=elem_size, stride_bytes_256=stride_bytes_256,
        gen_mode=0, single_packet=True, queue_num=queue_num,
        sbuf_tokens_per_rank=0, sbuf_free_dim_per_rank=0,
        sbuf_free_dim_pad_per_rank=0, sbuf_byte_offset=0))


def build_kernel(ncol, chunks, colmeta):
    total_cols = len(colmeta)
    nc = bacc.Bacc("TRN2", target_bir_lowering=False, debug=False,
                   num_devices=N_CORES, num_swdge_queues=4)

    xt_d = nc.dram_tensor("xt", [128, N_LOC], F32, kind="ExternalInput")
    idx_d = nc.dram_tensor("gidx", [128, total_cols * 8], I16,
                           kind="ExternalInput")
    dc_d = nc.dram_tensor("dcol", [128, total_cols], BF16,
                          kind="ExternalInput")
    degp_d = nc.dram_tensor("degp", [128, NBLK], F32, kind="ExternalInput")
    iota_d = nc.dram_tensor("iota", [128, 128], BF16, kind="ExternalInput")
    w1_d = nc.dram_tensor("w1", [128, CH], F32, kind="ExternalInput")
    w2_d = nc.dram_tensor("w2", [CH, CH], F32, kind="ExternalInput")
    w3_d = nc.dram_tensor("w3", [CH, CH], F32, kind="ExternalInput")
    w4_d = nc.dram_tensor("w4", [CH, 1], F32, kind="ExternalInput")
    bst_d = nc.dram_tensor("bst", [CH, 4], F32, kind="ExternalInput")
    gb_d = nc.dram_tensor("gbase", [G_MAX, 1], F32, kind="ExternalInput")
    cw1_d = nc.dram_tensor("cw1t", [97, 128], F32, kind="ExternalInput")
    cb1_d = nc.dram_tensor("cb1", [128, 1], F32, kind="ExternalInput")
    cw2_d = nc.dram_tensor("cw2t", [128, 5 * 64], F32, kind="ExternalInput")
    cb2_d = nc.dram_tensor("cb2", [64, 1], F32, kind="ExternalInput")
    fw1_d = nc.dram_tensor("fw1p", [64, 46 * 128], F32, kind="ExternalInput")
    fb1_d = nc.dram_tensor("fb1", [128, 1], F32, kind="ExternalInput")
    fw3_d = nc.dram_tensor("fw3t", [128, 2], F32, kind="ExternalInput")
    fb3_d = nc.dram_tensor("fb3", [2, 1], F32, kind="ExternalInput")
    out_d = nc.dram_tensor("out", [1, 2 * G_MAX], F32, kind="ExternalOutput")

    U32 = mybir.dt.uint32

    with tile.TileContext(nc) as tc:
        with tc.tile_pool(name="dram", bufs=1, space="DRAM") as dpool, \
             tc.tile_pool(name="const", bufs=1) as cpool, \
             tc.tile_pool(name="big", bufs=1) as bigpool, \
             tc.tile_pool(name="msg", bufs=4) as msgpool, \
             tc.tile_pool(name="mmat", bufs=3) as mpool, \
             tc.tile_pool(name="io", bufs=2) as iopool, \
             tc.tile_pool(name="work", bufs=2) as wpool, \
             tc.tile_pool(name="psA", bufs=2, space="PSUM") as psA, \
             tc.tile_pool(name="psB", bufs=2, space="PSUM") as psB, \
             tc.tile_pool(name="psS", bufs=2, space="PSUM") as psS:

            bounce = dpool.tile([N_LOC, CH], F32)
            tabC = dpool.tile([N_CORES * N_LOC, CH], F32)
            tabD = dpool.tile([NWIN * WIN_PAD, E_PAD], F32)
            xcd = dpool.tile([N_LOC, XC], F32)

            def load_const(dram, shape, dtype=F32):
                t = cpool.tile(shape, dtype, tag=f"c_{dram.name}")
                nc.sync.dma_start(out=t[:], in_=dram[:])
                return t

            w1 = load_const(w1_d, [128, CH])
            w2 = load_const(w2_d, [CH, CH])
            w3 = load_const(w3_d, [CH, CH])
            w4 = load_const(w4_d, [CH, 1])
            bst = load_const(bst_d, [CH, 4])
            gbase = load_const(gb_d, [G_MAX, 1])
            iota = load_const(iota_d, [128, 128], BF16)
            cw1t = load_const(cw1_d, [97, 128])
            cb1 = load_const(cb1_d, [128, 1])
            cw2t = load_const(cw2_d, [128, 5 * 64])
            cb2 = load_const(cb2_d, [64, 1])
            fw1p = load_const(fw1_d, [64, 46 * 128])
            fb1 = load_const(fb1_d, [128, 1])
            fw3t = load_const(fw3_d, [128, 2])
            fb3 = load_const(fb3_d, [2, 1])

            ident = cpool.tile([128, 128], F32)
            make_identity(nc, ident[:])

            # zero rows of each gather window (cols 0:CH suffice, but
            # clear the full padded row once)
            zrow = cpool.tile([128, E_PAD], F32)
            nc.vector.memset(zrow[:], 0.0)
            for q in range(NWIN):
                nc.sync.dma_start(
                    out=tabD[q * WIN_PAD + WIN_REAL:(q + 1) * WIN_PAD, :],
                    in_=zrow[:])

            disp = load_const(degp_d, [128, NBLK])
            nc.vector.reciprocal(out=disp[:], in_=disp[:])
            nc.scalar.activation(out=disp[:], in_=disp[:], func=ACT.Sqrt)

            hrows = bigpool.tile([128, NBLK, CH], F32)   # h' rows (scaled)
            xct = bigpool.tile([XC, N_LOC], F32)
            accR = bigpool.tile([128, NBLK, CH], F32)    # S accumulator
            gixA = bigpool.tile([128, total_cols * 8], I16)
            nc.sync.dma_start(out=gixA[:], in_=idx_d[:])
            dcolA = bigpool.tile([128, total_cols], BF16)
            nc.sync.dma_start(out=dcolA[:], in_=dc_d[:])

            # ---------------- pass prologue ----------------
            def prologue(pass_i):
                for ci in range(50):
                    sl = slice(ci * 256, (ci + 1) * 256)
                    tt = wpool.tile([CH, 256], F32, tag="htc")
                    if pass_i == 0:
                        xt_sb = wpool.tile([128, 256], F32, tag="xtc")
                        nc.sync.dma_start(out=xt_sb[:], in_=xt_d[:, sl])
                        ps = psA.tile([CH, 256], F32, space="PSUM", tag="ps")
                        nc.tensor.matmul(out=ps[:], lhsT=w1[:], rhs=xt_sb[:],
                                         start=True, stop=True)
                        nc.vector.tensor_copy(out=tt[:], in_=ps[:])
                    elif pass_i == 1:
                        ps = psA.tile([CH, 256], F32, space="PSUM", tag="ps")
                        nc.tensor.matmul(out=ps[:], lhsT=w2[:],
                                         rhs=xct[0:CH, sl],
                                         start=True, stop=True)
                        nc.vector.tensor_copy(out=tt[:], in_=ps[:])
                    else:
                        nc.vector.tensor_copy(out=tt[:], in_=xct[CH:2 * CH, sl])
                    for j in range(2):
                        c = 2 * ci + j
                        pst = psA.tile([128, CH], F32, space="PSUM", tag="ps")
                        nc.tensor.transpose(out=pst[:],
                                            in_=tt[:, 128 * j:128 * (j + 1)],
                                            identity=ident[0:CH, 0:CH])
                        nc.vector.tensor_scalar_mul(out=hrows[:, c, :],
                                                    in0=pst[:],
                                                    scalar1=disp[:, c:c + 1])
                nc.sync.dma_start(
                    out=bounce[:].rearrange("(b p) c -> p b c", p=128),
                    in_=hrows[:])

            # ---------------- epilogue for one block ----------------
            def block_epilogue(pass_i, b):
                sl = slice(b * 128, (b + 1) * 128)
                t1 = wpool.tile([128, CH], F32, tag="fl")
                nc.vector.tensor_tensor(out=t1[:], in0=accR[:, b, :],
                                        in1=hrows[:, b, :], op=ALU.add)
                nc.vector.tensor_scalar_mul(out=t1[:], in0=t1[:],
                                            scalar1=disp[:, b:b + 1])
                psu = psA.tile([CH, 128], F32, space="PSUM", tag="ps")
                nc.tensor.transpose(out=psu[:], in_=t1[:], identity=ident[:])
                if pass_i == 0:
                    nc.scalar.activation(out=xct[0:CH, sl], in_=psu[:],
                                         func=ACT.Tanh, bias=bst[:, 0:1])
                elif pass_i == 1:
                    nc.scalar.activation(out=xct[CH:2 * CH, sl], in_=psu[:],
                                         func=ACT.Tanh, bias=bst[:, 1:2])
                else:
                    ut = wpool.tile([CH, 128], F32, tag="ut")
                    nc.vector.tensor_copy(out=ut[:], in_=psu[:])
                    ps3 = psA.tile([CH, 128], F32, space="PSUM", tag="ps")
                    nc.tensor.matmul(out=ps3[:], lhsT=w3[:], rhs=ut[:],
                                     start=True, stop=True)
                    nc.scalar.activation(out=xct[2 * CH:3 * CH, sl],
                                         in_=ps3[:], func=ACT.Tanh,
                                         bias=bst[:, 2:3])
                    ps4 = psA.tile([1, 128], F32, space="PSUM", tag="ps")
                    nc.tensor.matmul(out=ps4[:], lhsT=w4[:], rhs=ut[:],
                                     start=True, stop=True)
                    nc.scalar.activation(out=xct[96:97, sl], in_=ps4[:],
                                         func=ACT.Tanh, bias=bst[0:1, 3:4])

            # ---------------- edge phase ----------------
            def edge_phase(pass_i):
                nc.gpsimd.collective_compute(
                    "AllGather", ALU.bypass,
                    replica_groups=[list(range(N_CORES))],
                    ins=[bounce[:]], outs=[tabC[:]])
                # restride 32ch rows -> 64ch padded windows
                for q in range(NWIN):
                    nc.sync.dma_start(
                        out=tabD[q * WIN_PAD:q * WIN_PAD + WIN_REAL, 0:CH],
                        in_=tabC[q * WIN_REAL:(q + 1) * WIN_REAL, :])

                nc.vector.memset(accR[:].rearrange("p b c -> p (b c)"), 0.0)

                seen = set()
                col = 0          # global column index
                mcur = None
                for ic, (q, ncols) in enumerate(chunks):
                    n = ncols * 128
                    mblk = msgpool.tile([128, CHUNK_COLS, CH], F32,
                                        tag="m")
                    if _ABL not in ("nogather", "skel"):
                        dma_gather_raw(
                            nc.gpsimd, mblk[:, 0:ncols, :],
                            tabD[q * WIN_PAD:(q + 1) * WIN_PAD, :],
                            gixA[:, col * 8:(col + ncols) * 8], n, CH, E_PAD,
                            queue_num=ic % 4)
                    if _ABL not in ("nomm", "skel"):
                        hl = msgpool.tile([128, CHUNK_COLS, 2, CH], BF16,
                                          tag="hl")
                        nc.vector.tensor_copy(out=hl[:, 0:ncols, 0, :],
                                              in_=mblk[:, 0:ncols, :])
                        nc.vector.tensor_tensor(out=mblk[:, 0:ncols, :],
                                                in0=mblk[:, 0:ncols, :],
                                                in1=hl[:, 0:ncols, 0, :],
                                                op=ALU.subtract)
                        nc.vector.tensor_copy(out=hl[:, 0:ncols, 1, :],
                                              in_=mblk[:, 0:ncols, :])
                    for j in range(ncols if _ABL not in ("nomm", "skel") else 0):
                        (qq, b, st, sp) = colmeta[col + j]
                        if j % MB == 0:
                            nb = min(MB, ncols - j)
                            mcur = mpool.tile([128, MB, 128], BF16, tag="mm")
                            nc.vector.tensor_tensor(
                                out=mcur[:, 0:nb, :],
                                in0=dcolA[:, col + j:col + j + nb].rearrange(
                                    "p (k o) -> p k o", o=1).to_broadcast(
                                    [128, nb, 128]),
                                in1=iota[:].rearrange(
                                    "p (o v) -> p o v", o=1).to_broadcast(
                                    [128, nb, 128]),
                                op=ALU.is_equal)
                        if st:
                            ps = psS.tile([128, 2 * CH], F32, space="PSUM",
                                          tag="s")
                            cur_ps = ps
                        nc.tensor.matmul(out=cur_ps[:],
                                         lhsT=mcur[:, j % MB, :],
                                         rhs=hl[:, j, :, :].rearrange(
                                             "p h c -> p (h c)"),
                                         start=st, stop=sp)
                        if sp:
                            if b in seen:
                                nc.vector.tensor_tensor(out=accR[:, b, :],
                                                        in0=accR[:, b, :],
                                                        in1=cur_ps[:, 0:CH],
                                                        op=ALU.add)
                            else:
                                nc.vector.tensor_copy(out=accR[:, b, :],
                                                      in_=cur_ps[:, 0:CH])
                                seen.add(b)
                            nc.vector.tensor_tensor(out=accR[:, b, :],
                                                    in0=accR[:, b, :],
                                                    in1=cur_ps[:, CH:2 * CH],
                                                    op=ALU.add)
                    col += ncols
                for b in range(NBLK):
                    block_epilogue(pass_i, b)

            for pass_i in range(3):
                prologue(pass_i)
                edge_phase(pass_i)

            # ---------------- xct -> xcd (rows in DRAM) ----------------
            for c in range(NBLK):
                pst = psA.tile([128, XC], F32, space="PSUM", tag="ps")
                nc.tensor.transpose(out=pst[:],
                                    in_=xct[:, c * 128:(c + 1) * 128],
                                    identity=ident[0:XC, 0:XC])
                xr = wpool.tile([128, XC], F32, tag="xr")
                nc.vector.tensor_copy(out=xr[:], in_=pst[:])
                nc.sync.dma_start(out=xcd[c * 128:(c + 1) * 128, :], in_=xr[:])

            # ---------------- sort pooling ----------------
            keys = cpool.tile([G_MAX, N_PER], F32)
            nc.sync.dma_start(
                out=keys[:],
                in_=xcd[:, 96:97].rearrange("(g k) c -> g k c", k=N_PER))
            vals = cpool.tile([G_MAX, N_PER], F32)
            nc.vector.tensor_copy(out=vals[:], in_=keys[:])
            idxs = cpool.tile([G_MAX, 104], U32)
            for r in range(13):
                m8 = wpool.tile([G_MAX, 8], F32, tag="m8")
                nc.vector.max(out=m8[:], in_=vals[:])
                nc.vector.max_index(out=idxs[:, 8 * r:8 * r + 8],
                                    in_max=m8[:], in_values=keys[:])
                nc.vector.match_replace(out=vals[:], in_to_replace=m8[:],
                                        in_values=vals[:], imm_value=-1e30)
            idxf = cpool.tile([G_MAX, 104], F32)
            nc.vector.tensor_copy(out=idxf[:], in_=idxs[:])
            nc.vector.tensor_scalar(out=idxf[:], in0=idxf[:],
                                    scalar1=gbase[:], scalar2=None,
                                    op0=ALU.add)
            psi = psA.tile([104, G_MAX], F32, space="PSUM", tag="ps")
            nc.tensor.transpose(out=psi[:], in_=idxf[:],
                                identity=ident[0:G_MAX, 0:G_MAX])
            idxT = cpool.tile([128, G_MAX], mybir.dt.int32)
            nc.vector.memset(idxT[:], 0)
            nc.vector.tensor_copy(out=idxT[0:104, :], in_=psi[:])

            # ---------------- head ----------------
            p1all = bigpool.tile([128, 50 * G_MAX], F32)
            for g in range(G_MAX):
                xsg = wpool.tile([128, XC], F32, tag="xsg")
                nc.gpsimd.indirect_dma_start(
                    out=xsg[:], out_offset=None, in_=xcd[:],
                    in_offset=bass.IndirectOffsetOnAxis(
                        ap=idxT[:, g:g + 1], axis=0))
                pst = psA.tile([XC, 104], F32, space="PSUM", tag="ps")
                nc.tensor.transpose(out=pst[:], in_=xsg[0:104, :],
                                    identity=ident[0:104, 0:104])
                xsgT = wpool.tile([XC, 104], F32, tag="xsgT")
                nc.vector.tensor_copy(out=xsgT[:], in_=pst[:])
                c1 = psB.tile([128, K], F32, space="PSUM", tag="acc")
                nc.tensor.matmul(out=c1[:], lhsT=cw1t[:],
                                 rhs=xsgT[0:97, 0:K], start=True, stop=True)
                mp = wpool.tile([128, 50], F32, tag="mp")
                nc.vector.tensor_copy(out=mp[:], in_=c1[:, 0:K:2])
                nc.vector.tensor_tensor(out=mp[:], in0=mp[:],
                                        in1=c1[:, 1:K:2], op=ALU.max)
                nc.scalar.activation(out=p1all[:, 50 * g:50 * g + 50],
                                     in_=mp[:], func=ACT.Relu, bias=cb1[:])

            c2all = bigpool.tile([64, 46 * G_MAX], F32)
            for g0 in range(0, G_MAX, 8):
                gn = min(8, G_MAX - g0)
                ps = psA.tile([64, 46 * gn], F32, space="PSUM", tag="ps")
                for t in range(5):
                    rhs = p1all[:].rearrange("p (g x) -> p g x", x=50)[
                        :, g0:g0 + gn, t:t + 46]
                    nc.tensor.matmul(out=ps[:],
                                     lhsT=cw2t[:, 64 * t:64 * (t + 1)],
                                     rhs=rhs, start=(t == 0), stop=(t == 4))
                nc.scalar.activation(out=c2all[:, 46 * g0:46 * (g0 + gn)],
                                     in_=ps[:], func=ACT.Relu, bias=cb2[:])

            hps = psA.tile([128, G_MAX], F32, space="PSUM", tag="ps")
            for p in range(46):
                rhs = c2all[:].rearrange("p (g x) -> p g x", x=46)[:, :, p:p + 1]
                nc.tensor.matmul(out=hps[:],
                                 lhsT=fw1p[:, 128 * p:128 * (p + 1)],
                                 rhs=rhs, start=(p == 0), stop=(p == 45))
            hsb = cpool.tile([128, G_MAX], F32)
            nc.scalar.activation(out=hsb[:], in_=hps[:], func=ACT.Relu,
                                 bias=fb1[:])

            lps = psA.tile([2, G_MAX], F32, space="PSUM", tag="ps")
            nc.tensor.matmul(out=lps[:], lhsT=fw3t[:], rhs=hsb[:],
                             start=True, stop=True)
            lg2 = cpool.tile([2, G_MAX], F32)
            nc.vector.tensor_copy(out=lg2[:], in_=lps[:])
            lgf = cpool.tile([1, 2 * G_MAX], F32)
            nc.sync.dma_start(
                out=lgf[:].rearrange("p (c g) -> p c g", c=2), in_=lg2[:])
            fb3f = cpool.tile([1, 2], F32)
            nc.sync.dma_start(out=fb3f[:], in_=fb3_d[:].rearrange("c x -> x c"))
            G = G_MAX
            for c in range(2):
                nc.vector.tensor_tensor(
                    out=lgf[:, c * G:(c + 1) * G],
                    in0=lgf[:, c * G:(c + 1) * G],
                    in1=fb3f[:, c:c + 1].to_broadcast([1, G]), op=ALU.add)
            mx = cpool.tile([1, G], F32)
            nc.vector.tensor_tensor(out=mx[:], in0=lgf[:, 0:G],
                                    in1=lgf[:, G:2 * G], op=ALU.max)
            dd = cpool.tile([1, 2 * G], F32)
            for c in range(2):
                nc.vector.tensor_tensor(out=dd[:, c * G:(c + 1) * G],
                                        in0=lgf[:, c * G:(c + 1) * G],
                                        in1=mx[:], op=ALU.subtract)
            ee = cpool.tile([1, 2 * G], F32)
            nc.scalar.activation(out=ee[:], in_=dd[:], func=ACT.Exp)
            ss = cpool.tile([1, G], F32)
            nc.vector.tensor_tensor(out=ss[:], in0=ee[:, 0:G],
                                    in1=ee[:, G:2 * G], op=ALU.add)
            nc.scalar.activation(out=ss[:], in_=ss[:], func=ACT.Ln)
            res = cpool.tile([1, 2 * G], F32)
            for c in range(2):
                nc.vector.tensor_tensor(out=res[:, c * G:(c + 1) * G],
                                        in0=dd[:, c * G:(c + 1) * G],
                                        in1=ss[:], op=ALU.subtract)
            nc.sync.dma_start(out=out_d[:], in_=res[:])

    nc.compile()
    return nc


_CACHED = {}


def kernel(**inputs):
    x = np.asarray(inputs["x"], np.float32)
    edge_index = np.asarray(inputs["edge_index"])
    (idx_cores, dc_cores, deg_cores, ncol, chunks, colmeta,
     struct) = host_prep(edge_index)

    W = {k: np.asarray(inputs[k], np.float32) for k in
         ("W1", "b1", "W2", "b2", "W3", "b3", "W4", "b4",
          "cw1", "cb1", "cw2", "cb2", "fw1", "fb1", "fw3", "fb3")}

    if _CACHED.get("key") != struct:
        _CACHED["nc"] = build_kernel(ncol, chunks, colmeta)
        _CACHED["key"] = struct
    nc = _CACHED["nc"]

    bs = np.zeros((4, CH), np.float32)
    bs[0], bs[1], bs[2] = W["b1"], W["b2"], W["b3"]
    bs[3, 0] = W["b4"][0]
    bst = np.ascontiguousarray(bs.T)  # [CH, 4]
    cw2t = W["cw2"].transpose(1, 2, 0)            # [ic, t, oc]
    cw2t = np.ascontiguousarray(cw2t.reshape(128, 5 * 64))
    fw1p = W["fw1"].reshape(128, 64, 46).transpose(1, 2, 0)  # [ch,pos,hid]
    fw1p = np.ascontiguousarray(fw1p.reshape(64, 46 * 128))
    shared = {
        "w1": W["W1"], "w2": W["W2"], "w3": W["W3"], "w4": W["W4"],
        "bst": bst,
        "gbase": (np.arange(G_MAX, dtype=np.float32) * N_PER)[:, None],
        "iota": np.tile(np.arange(128, dtype=np.float32)[None, :], (128, 1)),
        # iota/dcol ship as bf16 (exact for ints < 256)
        "cw1t": np.ascontiguousarray(W["cw1"][:, 0, :].T),
        "cb1": W["cb1"][:, None],
        "cw2t": cw2t, "cb2": W["cb2"][:, None],
        "fw1p": fw1p, "fb1": W["fb1"][:, None],
        "fw3t": np.ascontiguousarray(W["fw3"].T), "fb3": W["fb3"][:, None],
    }

    in_maps = []
    for c in range(N_CORES):
        lo = NODE_BASE[c]
        xc_ = np.zeros((N_LOC, 128), np.float32)
        xc_[: N_REAL[c]] = x[lo: lo + N_REAL[c]]
        m = {
            "xt": np.ascontiguousarray(xc_.T),
            "gidx": idx_cores[c],
            "dcol": dc_cores[c],
            "degp": np.ascontiguousarray(deg_cores[c].reshape(NBLK, 128).T),
        }
        m.update(shared)
        import ml_dtypes
        bf = ml_dtypes.bfloat16
        in_maps.append({k: np.ascontiguousarray(
            v, dtype=np.int16 if k == "gidx" else
            bf if k in ("dcol", "iota") else np.float32)
            for k, v in m.items()})

    _CACHED["in_maps"] = in_maps
    res = run_bass_kernel_spmd(nc, in_maps, core_ids=list(range(N_CORES)))

    out = np.zeros((NUM_GRAPHS, 2), np.float32)
    gb = 0
    for c in range(N_CORES):
        g = GRAPHS_PER_CORE[c]
        out[gb:gb + g] = res.results[c]["out"].reshape(2, G_MAX)[:, :g].T
        gb += g
    return out
